# revision 10
# baseline (speedup 1.0000x reference)
"""Temporal-shift + 1x1 conv (TSM block) Trainium2 kernel.

Full problem: x [128, 256, 28, 28] f32 (16 clips x 8 frames), net_weight
[256, 256] f32.  out[n,o,h,w] = sum_c W[o,c] * shift(x)[n,c,h,w] where
shift moves channels 0:32 forward in time (out[t] = x[t-1]) and channels
32:64 backward (out[t] = x[t+1]) within each 8-frame clip.

Sharding: data-parallel over clips — each of 8 cores takes 2 clips
(16 consecutive frames).  The shift never crosses clip boundaries, so no
halo exchange; the weight is replicated.

Device kernel (per core): the shift is folded into the input DMAs — the
rhs chunk for channels 0:128 is composed from three DMA slices (prev
frame's ch 0:32, next frame's ch 32:64, own ch 64:128) with memset for
clip-boundary frames.  GEMM: out[o, pix] = wT[c, o].T @ y[c, pix] on the
PE, accumulating the two 128-channel K-chunks in PSUM.
"""

import sys

for _p in ("/opt/trn_rl_repo", "/opt/pypackages"):
    if _p not in sys.path:
        sys.path.append(_p)

import numpy as np

import concourse.bass as bass
import concourse.mybir as mybir
import concourse.bacc as bacc
import concourse.tile as tile
from concourse.bass_utils import run_bass_kernel_spmd

# ---- problem constants (hardcoded; kernel.py must be self-contained) ----
NT, C, H, W = 128, 256, 28, 28
N_SEGMENT = 8            # frames per clip
FOLD = C // 8            # 32 channels shift each way
N_CORES = 8
FPC = NT // N_CORES      # 16 frames per core (2 clips)
PIX = H * W              # 784
F = 4                    # frames per super-tile
N_ST = FPC // F          # 4 super-tiles per core
FREE = F * PIX           # 3136 moving-dim elements per super-tile
N_TILE = 448             # matmul moving tile (784*4 = 7*448, 448*4B < 2KB PSUM bank)
N_CHUNKS = FREE // N_TILE  # 7
KC = C // 128            # 2 contraction chunks
MC = C // 128            # 2 output-channel chunks

F32 = mybir.dt.float32
# float32r: PE reduced-precision fp32 path, 1 cycle/row at N>=256 (vs 4 for
# exact fp32).  Flip to False if accuracy requires exact fp32 matmuls.
USE_F32R = True
MM_DT = mybir.dt.float32r if USE_F32R else F32


def build_kernel() -> bacc.Bacc:
    nc = bacc.Bacc("TRN2", target_bir_lowering=False, debug=False,
                   num_devices=N_CORES)

    x = nc.dram_tensor("x", [FPC, C, H, W], MM_DT, kind="ExternalInput").ap()
    wT = nc.dram_tensor("wT", [C, C], MM_DT, kind="ExternalInput").ap()
    out = nc.dram_tensor("out", [FPC, C, H, W], F32, kind="ExternalOutput").ap()

    with tile.TileContext(nc) as tc:
        with (
            tc.tile_pool(name="wpool", bufs=1) as wpool,
            tc.tile_pool(name="inpool", bufs=3) as inpool,
            tc.tile_pool(name="outpool", bufs=2) as outpool,
            tc.tile_pool(name="psum", bufs=1, space="PSUM") as psum,
        ):
            # weights: wt[c', k, o] = wT[k*128 + c', o]
            wt = wpool.tile([128, KC, C], MM_DT)
            nc.sync.dma_start(wt[:], wT.rearrange("(k c) o -> c k o", k=KC))

            for t in range(N_ST):
                f0 = t * F
                s = f0 % N_SEGMENT  # start frame within its clip (0 or 4)

                # rhs chunk 0 (channels 0:128, shift composed) and chunk 1
                in0 = inpool.tile([128, F, PIX], MM_DT)
                in1 = inpool.tile([128, F, PIX], MM_DT)

                def src(fa, fb, ca, cb):
                    return x[fa:fb, ca:cb].rearrange("f c h w -> c f (h w)")

                # channels 0:32 <- prev frame (zero at clip start)
                if s == 0:
                    nc.gpsimd.memset(in0[0:FOLD, 0, :].bitcast(F32), 0.0)
                    nc.sync.dma_start(in0[0:FOLD, 1:F, :],
                                      src(f0, f0 + F - 1, 0, FOLD))
                else:
                    nc.sync.dma_start(in0[0:FOLD, :, :],
                                      src(f0 - 1, f0 + F - 1, 0, FOLD))
                # channels 32:64 <- next frame (zero at clip end)
                if s + F == N_SEGMENT:
                    nc.sync.dma_start(in0[FOLD:2 * FOLD, 0:F - 1, :],
                                      src(f0 + 1, f0 + F, FOLD, 2 * FOLD))
                    nc.gpsimd.memset(in0[FOLD:2 * FOLD, F - 1, :].bitcast(F32), 0.0)
                else:
                    nc.sync.dma_start(in0[FOLD:2 * FOLD, :, :],
                                      src(f0 + 1, f0 + F + 1, FOLD, 2 * FOLD))
                # channels 64:128 unshifted
                nc.sync.dma_start(in0[2 * FOLD:128, :, :],
                                  src(f0, f0 + F, 2 * FOLD, 128))
                # channels 128:256 unshifted
                nc.sync.dma_start(in1[:], src(f0, f0 + F, 128, 256))

                rhs = [in0[:].rearrange("c f p -> c (f p)"),
                       in1[:].rearrange("c f p -> c (f p)")]

                for m in range(MC):
                    om = outpool.tile([128, FREE], F32)
                    ps = [psum.tile([128, N_TILE], F32, name=f"ps{n}",
                                    tag=f"ps{n}")
                          for n in range(N_CHUNKS)]
                    # k-outer keeps the stationary operand fixed across the
                    # 7 moving tiles -> minimal weight reloads
                    for k in range(KC):
                        lhsT = wt[:, k, m * 128:(m + 1) * 128]
                        for n in range(N_CHUNKS):
                            nc.tensor.matmul(
                                ps[n][:],
                                lhsT,
                                rhs[k][:, n * N_TILE:(n + 1) * N_TILE],
                                start=(k == 0), stop=(k == KC - 1))
                    for n in range(N_CHUNKS):
                        nc.vector.tensor_copy(
                            om[:, n * N_TILE:(n + 1) * N_TILE], ps[n][:])
                    nc.scalar.dma_start(
                        out[f0:f0 + F, m * 128:(m + 1) * 128]
                        .rearrange("f c h w -> c f (h w)"),
                        om[:].rearrange("c (f p) -> c f p", p=PIX))

    nc.compile()
    return nc


_NC_CACHE = None


def _get_nc():
    global _NC_CACHE
    if _NC_CACHE is None:
        _NC_CACHE = build_kernel()
    return _NC_CACHE


def run(x: np.ndarray, net_weight: np.ndarray, **spmd_kwargs):
    """Returns (out, BassKernelResults)."""
    nc = _get_nc()
    x = np.ascontiguousarray(x, dtype=np.float32)
    wT = np.ascontiguousarray(net_weight.T, dtype=np.float32)
    in_maps = [
        {"x": x[i * FPC:(i + 1) * FPC], "wT": wT} for i in range(N_CORES)
    ]
    res = run_bass_kernel_spmd(nc, in_maps, core_ids=list(range(N_CORES)),
                               **spmd_kwargs)
    out = np.concatenate([res.results[i]["out"] for i in range(N_CORES)],
                         axis=0)
    return out, res


def kernel(x: np.ndarray, net_weight: np.ndarray) -> np.ndarray:
    out, _ = run(x, net_weight)
    return out


if __name__ == "__main__":
    xs = np.random.randn(NT, C, H, W).astype(np.float32)
    ws = (np.random.randn(C, C) * 0.0625).astype(np.float32)
    o = kernel(xs, ws)
    print("out", o.shape, o.dtype, float(np.abs(o).max()))


# revision 12
# speedup vs baseline: 1.0117x; 1.0117x over previous
"""Temporal-shift + 1x1 conv (TSM block) Trainium2 kernel.

Full problem: x [128, 256, 28, 28] f32 (16 clips x 8 frames), net_weight
[256, 256] f32.  out[n,o,h,w] = sum_c W[o,c] * shift(x)[n,c,h,w] where
shift moves channels 0:32 forward in time (out[t] = x[t-1]) and channels
32:64 backward (out[t] = x[t+1]) within each 8-frame clip.

Sharding: data-parallel over clips — each of 8 cores takes 2 clips
(16 consecutive frames).  The shift never crosses clip boundaries, so no
halo exchange; the weight is replicated.

Device kernel (per core): the shift is folded into the input DMAs — the
rhs chunk for channels 0:128 is composed from three DMA slices (prev
frame's ch 0:32, next frame's ch 32:64, own ch 64:128) with memset for
clip-boundary frames.  GEMM: out[o, pix] = wT[c, o].T @ y[c, pix] on the
PE, accumulating the two 128-channel K-chunks in PSUM.
"""

import sys

for _p in ("/opt/trn_rl_repo", "/opt/pypackages"):
    if _p not in sys.path:
        sys.path.append(_p)

import numpy as np

import concourse.bass as bass
import concourse.mybir as mybir
import concourse.bacc as bacc
import concourse.tile as tile
from concourse.bass_utils import run_bass_kernel_spmd

# ---- problem constants (hardcoded; kernel.py must be self-contained) ----
NT, C, H, W = 128, 256, 28, 28
N_SEGMENT = 8            # frames per clip
FOLD = C // 8            # 32 channels shift each way
N_CORES = 8
FPC = NT // N_CORES      # 16 frames per core (2 clips)
PIX = H * W              # 784
F = 4                    # frames per super-tile (max)
# (start_frame, n_frames) per super-tile; must not straddle clip bounds
SUPER_TILES = [(0, 4), (4, 4), (8, 4), (12, 4)]
N_TILE = 392             # matmul moving tile (2 per frame; 392*4B < 2KB PSUM bank)
CPF = PIX // N_TILE      # 2 psum chunks per frame
KC = C // 128            # 2 contraction chunks
MC = C // 128            # 2 output-channel chunks

F32 = mybir.dt.float32
# float32r: PE reduced-precision fp32 path, 1 cycle/row at N>=256 (vs 4 for
# exact fp32).  Flip to False if accuracy requires exact fp32 matmuls.
USE_F32R = True
MM_DT = mybir.dt.float32r if USE_F32R else F32


def build_kernel() -> bacc.Bacc:
    nc = bacc.Bacc("TRN2", target_bir_lowering=False, debug=False,
                   num_devices=N_CORES)

    x = nc.dram_tensor("x", [FPC, C, H, W], MM_DT, kind="ExternalInput").ap()
    wT = nc.dram_tensor("wT", [C, C], MM_DT, kind="ExternalInput").ap()
    out = nc.dram_tensor("out", [FPC, C, H, W], F32, kind="ExternalOutput").ap()

    with tile.TileContext(nc) as tc:
        with (
            tc.tile_pool(name="wpool", bufs=1) as wpool,
            tc.tile_pool(name="inpool", bufs=4) as inpool,
            tc.tile_pool(name="outpool", bufs=3) as outpool,
            tc.tile_pool(name="psum", bufs=1, space="PSUM") as psum,
        ):
            # weights: wt[c', k, o] = wT[k*128 + c', o]
            wt = wpool.tile([128, KC, C], MM_DT)
            nc.sync.dma_start(wt[:], wT.rearrange("(k c) o -> c k o", k=KC))

            def src(fa, fb, ca, cb):
                return x[fa:fb, ca:cb].rearrange("f c h w -> c f (h w)")

            for f0, nf in SUPER_TILES:
                s = f0 % N_SEGMENT   # start frame within its clip
                free = nf * PIX
                nck = nf * CPF       # psum chunks this super-tile

                # rhs chunk 0 (channels 0:128, shift composed) and chunk 1
                in0 = inpool.tile([128, F, PIX], MM_DT)
                in1 = inpool.tile([128, F, PIX], MM_DT)

                # channels 0:32 <- prev frame (zero at clip start)
                if s == 0:
                    nc.gpsimd.memset(in0[0:FOLD, 0, :].bitcast(F32), 0.0)
                    nc.sync.dma_start(in0[0:FOLD, 1:nf, :],
                                      src(f0, f0 + nf - 1, 0, FOLD))
                else:
                    nc.sync.dma_start(in0[0:FOLD, 0:nf, :],
                                      src(f0 - 1, f0 + nf - 1, 0, FOLD))
                # channels 32:64 <- next frame (zero at clip end)
                if s + nf == N_SEGMENT:
                    nc.sync.dma_start(in0[FOLD:2 * FOLD, 0:nf - 1, :],
                                      src(f0 + 1, f0 + nf, FOLD, 2 * FOLD))
                    nc.gpsimd.memset(
                        in0[FOLD:2 * FOLD, nf - 1, :].bitcast(F32), 0.0)
                else:
                    nc.sync.dma_start(in0[FOLD:2 * FOLD, 0:nf, :],
                                      src(f0 + 1, f0 + nf + 1, FOLD, 2 * FOLD))
                # channels 64:128 unshifted
                nc.sync.dma_start(in0[2 * FOLD:128, 0:nf, :],
                                  src(f0, f0 + nf, 2 * FOLD, 128))
                # channels 128:256 unshifted
                nc.sync.dma_start(in1[:, 0:nf, :], src(f0, f0 + nf, 128, 256))

                rhs = [in0[:].rearrange("c f p -> c (f p)"),
                       in1[:].rearrange("c f p -> c (f p)")]

                for m in range(MC):
                    ms = slice(m * 128, (m + 1) * 128)
                    # two half-tile output buffers -> store starts earlier
                    oma = outpool.tile([128, (F // 2) * PIX], F32)
                    omb = outpool.tile([128, (F // 2) * PIX], F32)
                    om = [oma, omb]
                    ps = [psum.tile([128, N_TILE], F32, name=f"ps{n}",
                                    tag=f"ps{n}")
                          for n in range(nck)]
                    # k-outer keeps the stationary operand fixed across the
                    # moving tiles -> minimal weight reloads
                    for k in range(KC):
                        lhsT = wt[:, k, ms]
                        for n in range(nck):
                            nc.tensor.matmul(
                                ps[n][:],
                                lhsT,
                                rhs[k][:, n * N_TILE:(n + 1) * N_TILE],
                                start=(k == 0), stop=(k == KC - 1))
                    # copies + half stores: chunks [0, nck/2) -> oma,
                    # [nck/2, nck) -> omb
                    half = nck // 2
                    fh = nf // 2
                    for h in range(2):
                        for j in range(half):
                            n = h * half + j
                            nc.vector.tensor_copy(
                                om[h][:, j * N_TILE:(j + 1) * N_TILE],
                                ps[n][:])
                        nc.scalar.dma_start(
                            out[f0 + h * fh:f0 + (h + 1) * fh, ms]
                            .rearrange("f c h w -> c f (h w)"),
                            om[h][:, 0:fh * PIX]
                            .rearrange("c (f p) -> c f p", p=PIX))

    nc.compile()
    return nc


_NC_CACHE = None


def _get_nc():
    global _NC_CACHE
    if _NC_CACHE is None:
        _NC_CACHE = build_kernel()
    return _NC_CACHE


def run(x: np.ndarray, net_weight: np.ndarray, **spmd_kwargs):
    """Returns (out, BassKernelResults)."""
    nc = _get_nc()
    x = np.ascontiguousarray(x, dtype=np.float32)
    wT = np.ascontiguousarray(net_weight.T, dtype=np.float32)
    in_maps = [
        {"x": x[i * FPC:(i + 1) * FPC], "wT": wT} for i in range(N_CORES)
    ]
    res = run_bass_kernel_spmd(nc, in_maps, core_ids=list(range(N_CORES)),
                               **spmd_kwargs)
    out = np.concatenate([res.results[i]["out"] for i in range(N_CORES)],
                         axis=0)
    return out, res


def kernel(x: np.ndarray, net_weight: np.ndarray) -> np.ndarray:
    out, _ = run(x, net_weight)
    return out


if __name__ == "__main__":
    xs = np.random.randn(NT, C, H, W).astype(np.float32)
    ws = (np.random.randn(C, C) * 0.0625).astype(np.float32)
    o = kernel(xs, ws)
    print("out", o.shape, o.dtype, float(np.abs(o).max()))
